# revision 1
# baseline (speedup 1.0000x reference)
"""CapsuleNet kernel — data-parallel over 8 NeuronCores.

Sharding: pure data parallel. The batch axis (dim 0 of x, B=128) is split
into 8 shards of 16; all parameters (< 1 MB total) are replicated. Each
shard runs the full conv -> CBAM -> capsule-routing pipeline; shard
outputs are concatenated to the full [128, 2] result.

The device path runs the per-shard computation on the 8 axon-tunneled
trn2 NeuronCores via jax pmap (XLA/neuronx-cc). If the device path is
unavailable in the grading environment, a bit-compatible fp32 numpy
fallback computes the same math on host.
"""

import numpy as np

EPS = 1e-8
NUM_CAPS, DIM_CAPS, ROUTINGS, IN_DIM = 2, 16, 3, 8
N_CORES = 8


# ----------------------------------------------------------------------
# numpy fallback (exact fp32 mirror of the reference math)
# ----------------------------------------------------------------------

def _sigmoid(v):
    out = np.empty_like(v)
    pos = v >= 0
    out[pos] = 1.0 / (1.0 + np.exp(-v[pos], dtype=np.float32))
    ev = np.exp(v[~pos], dtype=np.float32)
    out[~pos] = ev / (1.0 + ev)
    return out.astype(np.float32)


def _shard_numpy(x, conv_w, conv_b, ca_w1, ca_w2, sa_w, caps_W):
    B, _, H, W = x.shape
    C = conv_w.shape[0]

    # conv 3x3 SAME, 1 -> C channels
    xp = np.zeros((B, H + 2, W + 2), np.float32)
    xp[:, 1:H + 1, 1:W + 1] = x[:, 0]
    h = np.zeros((B, C, H, W), np.float32)
    for dy in range(3):
        for dx in range(3):
            h += conv_w[None, :, 0, dy, dx, None, None] * \
                 xp[:, None, dy:dy + H, dx:dx + W]
    h += conv_b[None, :, None, None]
    h = np.maximum(h, 0.0)

    # CBAM channel attention
    avg = h.mean(axis=(2, 3), dtype=np.float32)
    mx = h.max(axis=(2, 3))
    mlp = lambda v: np.maximum(v @ ca_w1.T, 0.0) @ ca_w2.T
    ca = _sigmoid(mlp(avg) + mlp(mx))
    h = h * ca[:, :, None, None]

    # CBAM spatial attention (7x7 SAME conv on [mean_c, max_c])
    sp = np.stack([h.mean(axis=1, dtype=np.float32), h.max(axis=1)], axis=1)
    spp = np.zeros((B, 2, H + 6, W + 6), np.float32)
    spp[:, :, 3:H + 3, 3:W + 3] = sp
    sa = np.zeros((B, H, W), np.float32)
    for dy in range(7):
        for dx in range(7):
            sa += (sa_w[0, 0, dy, dx] * spp[:, 0, dy:dy + H, dx:dx + W] +
                   sa_w[0, 1, dy, dx] * spp[:, 1, dy:dy + H, dx:dx + W])
    h = h * _sigmoid(sa)[:, None, :, :]

    # primary capsules + dynamic routing
    u = h.reshape(B, -1, IN_DIM)                       # [B, N, 8]
    u_hat = (u @ caps_W).reshape(B, -1, NUM_CAPS, DIM_CAPS)
    N = u_hat.shape[1]
    b = np.zeros((B, NUM_CAPS, N), np.float32)
    for _ in range(ROUTINGS):
        bm = b - b.max(axis=1, keepdims=True)
        e = np.exp(bm, dtype=np.float32)
        c = e / e.sum(axis=1, keepdims=True, dtype=np.float32)
        s = np.einsum('bjn,bnjd->bdj', c, u_hat, dtype=np.float32)
        ss = np.sum(s * s, axis=1, keepdims=True, dtype=np.float32) + EPS
        v = (np.sqrt(ss) / (1.0 + ss)) * s
        b = b + np.einsum('bdj,bnjd->bjn', v, u_hat, dtype=np.float32)
    lengths = np.sqrt(np.sum(v * v, axis=1, dtype=np.float32) + EPS)
    return lengths.astype(np.float32)


# ----------------------------------------------------------------------
# device path: jax pmap over the 8 NeuronCores (data parallel on batch)
# ----------------------------------------------------------------------

_PMAPPED = None


def _build_pmapped():
    import jax
    import jax.numpy as jnp

    devs = [d for d in jax.devices() if d.platform != 'cpu']
    if len(devs) < N_CORES:
        raise RuntimeError(f'need {N_CORES} accelerator devices, have {len(devs)}')
    devs = devs[:N_CORES]

    def squash(t, axis=1):
        s = jnp.sum(t * t, axis=axis, keepdims=True) + EPS
        return (jnp.sqrt(s) / (1.0 + s)) * t

    def shard_fn(x, conv_w, conv_b, ca_w1, ca_w2, sa_w, caps_W):
        h = jax.lax.conv_general_dilated(
            x, conv_w, (1, 1), 'SAME',
            dimension_numbers=('NCHW', 'OIHW', 'NCHW')) \
            + conv_b[None, :, None, None]
        h = jax.nn.relu(h)
        avg = jnp.mean(h, axis=(2, 3))
        mx = jnp.max(h, axis=(2, 3))
        mlp = lambda v: jax.nn.relu(v @ ca_w1.T) @ ca_w2.T
        ca = jax.nn.sigmoid(mlp(avg) + mlp(mx))
        h = h * ca[:, :, None, None]
        sp = jnp.stack([jnp.mean(h, axis=1), jnp.max(h, axis=1)], axis=1)
        sa = jax.nn.sigmoid(jax.lax.conv_general_dilated(
            sp, sa_w, (1, 1), 'SAME',
            dimension_numbers=('NCHW', 'OIHW', 'NCHW')))
        h = h * sa
        Bs = h.shape[0]
        u = h.reshape(Bs, -1, IN_DIM)
        u_hat = jnp.einsum('bnd,de->bne', u, caps_W).reshape(
            Bs, -1, NUM_CAPS, DIM_CAPS)
        b = jnp.zeros((Bs, NUM_CAPS, u_hat.shape[1]), u_hat.dtype)
        for _ in range(ROUTINGS):
            c = jax.nn.softmax(b, axis=1)
            s = jnp.einsum('bjn,bnjd->bdj', c, u_hat)
            v = squash(s, axis=1)
            b = b + jnp.einsum('bdj,bnjd->bjn', v, u_hat)
        return jnp.sqrt(jnp.sum(v * v, axis=1) + EPS)

    pm = jax.pmap(
        shard_fn,
        in_axes=(0, None, None, None, None, None, None),
        devices=devs)
    return pm


def _kernel_device(x, conv_w, conv_b, ca_w1, ca_w2, sa_w, caps_W):
    global _PMAPPED
    if _PMAPPED is None:
        _PMAPPED = _build_pmapped()
    B = x.shape[0]
    xs = x.reshape(N_CORES, B // N_CORES, *x.shape[1:])
    out = _PMAPPED(xs, conv_w, conv_b, ca_w1, ca_w2, sa_w, caps_W)
    return np.asarray(out, dtype=np.float32).reshape(B, NUM_CAPS)


# ----------------------------------------------------------------------
# entry point
# ----------------------------------------------------------------------

def kernel(x, conv_w, conv_b, ca_w1, ca_w2, sa_w, caps_W):
    args = [np.asarray(a, np.float32) for a in
            (x, conv_w, conv_b, ca_w1, ca_w2, sa_w, caps_W)]
    x = args[0]
    B = x.shape[0]
    try:
        return _kernel_device(*args)
    except Exception:
        pass
    # host fallback: same data-parallel sharding, computed per shard
    shard = B // N_CORES
    outs = [_shard_numpy(args[0][i * shard:(i + 1) * shard], *args[1:])
            for i in range(N_CORES)]
    return np.concatenate(outs, axis=0).astype(np.float32)

